# revision 15
# baseline (speedup 1.0000x reference)
"""Multi-head causal self-attention (B=2, S=2048, D=1024, H=16) on 8 TRN2 NeuronCores.

Sharding: data-parallel over batch (2) x tensor-parallel over heads (4 groups of
4 heads). Each core computes Q/K/V projections for its 4 heads, causal
flash-style attention (scores kept transposed [k, q] so no on-chip transposes
are needed), and a partial output projection against its row-slice of W_O.
Host sums the 4 partials per batch and adds the output bias.

v2: single software-pipelined program. Attention kb-steps (QK pair -> staircase
add -> exp -> PV pair) form the backbone; projection tiles and output-projection
tiles are interleaved into the PE stream as filler so the PE never idles (keeps
the HAM clock gate at full rate). Softmax denominators come from an all-ones
column appended to V; 1/den = exp(-ln(den)) on ACT (single pinned activation
table: natural_log_exp_and_others), broadcast across partitions on the idle
GPSIMD engine (no DRAM bounce), multiplied on DVE. Q/K biases are folded into
the PSUM evacuations as per-partition scalars. DMA issues are spread across
four engine sequencers so descriptor-generation latency doesn't gate startup.
"""

import contextlib
import sys
from collections import deque

import ml_dtypes
import numpy as np

sys.path.insert(0, "/opt/trn_rl_repo")

import bass_rust as _bass_rust  # noqa: E402
import concourse.bass as bass  # noqa: E402,F401
import concourse.tile as tile  # noqa: E402
from concourse import bacc, mybir  # noqa: E402
from concourse.bass_utils import run_bass_kernel_spmd  # noqa: E402
from concourse.hw_specs import get_activation_tables  # noqa: E402

F32 = mybir.dt.float32
F32R = mybir.dt.float32r
BF16 = mybir.dt.bfloat16
AF = mybir.ActivationFunctionType

B, S, D, H = 2, 2048, 1024, 16
DH = D // H          # 64
TPG = 4              # tensor-parallel groups
HPC = H // TPG       # 4 heads per core
CH = HPC * DH        # 256 channels per core
CHA = CH + HPC       # 260: V channels augmented with a ones column per head
NEG = -1.0e9
N_CORES = 8
NQ = S // 512        # 4 q-blocks of 512
NT = S // 128        # 16 s-tiles / k-blocks

ACT_TABLE = "natural_log_exp_and_others"  # ln + exp + copy in one table

_PROG = None  # cached compiled Bass program


def _pin_act_table(nc):
    """Constrain the activation-table chooser to ACT_TABLE so ln/exp/copy
    never thrash between table sets (each reload is 1283ns on the ACT
    engine, and lands on the softmax critical path). Table ids stay index-
    aligned with act_info.json; non-target sets are emptied so first-match
    lands on the combined table."""
    tables = [
        (name, (funcs if name == ACT_TABLE else set()))
        for name, funcs in get_activation_tables(nc.m.arch).items()
    ]
    assert any(name == ACT_TABLE and funcs for name, funcs in tables)
    nc.insert_act_table_loads = (
        lambda: _bass_rust.insert_act_table_loads(nc, tables))


def _build_program():
    nc = bacc.Bacc("TRN2", target_bir_lowering=False, debug=False,
                   num_devices=N_CORES)
    _pin_act_table(nc)

    xT = nc.dram_tensor("xT", [D, S], BF16, kind="ExternalInput").ap()
    wq = nc.dram_tensor("wq", [D, CH], BF16, kind="ExternalInput").ap()
    wk = nc.dram_tensor("wk", [D, CH], BF16, kind="ExternalInput").ap()
    wv = nc.dram_tensor("wv", [D, CHA], BF16, kind="ExternalInput").ap()
    wo = nc.dram_tensor("wo", [CH, D], BF16, kind="ExternalInput").ap()
    bq = nc.dram_tensor("bq", [128, 2], F32, kind="ExternalInput").ap()
    bk = nc.dram_tensor("bk", [128, 2], F32, kind="ExternalInput").ap()
    bv = nc.dram_tensor("bv", [1, CHA], BF16, kind="ExternalInput").ap()
    tri = nc.dram_tensor("tri", [128, 512], F32, kind="ExternalInput").ap()
    ones = nc.dram_tensor("ones", [1, 512], BF16, kind="ExternalInput").ap()
    onesf = nc.dram_tensor("onesf", [1, 64], F32, kind="ExternalInput").ap()
    out = nc.dram_tensor("out", [S, D], BF16, kind="ExternalOutput").ap()

    with tile.TileContext(nc) as tc, contextlib.ExitStack() as ctx:
        const = ctx.enter_context(tc.tile_pool(name="const", bufs=1))
        xt = const.tile([128, 8, S], BF16)
        wq_t = const.tile([128, 8, CH], BF16)
        wk_t = const.tile([128, 8, CH], BF16)
        wv_t = const.tile([128, 8, CHA], BF16)
        qt = const.tile([128, 2, S], F32R)     # Q^T/8 (+bq/8): rows 0-63 even head
        kt = const.tile([128, 2, S], F32R)     # K^T (+bk)
        va = const.tile([128, NT, CHA], BF16)  # V augmented, head-major 65-col blocks
        otn = const.tile([128, 2, S], BF16)    # normalized attention out, transposed
        wo_t = const.tile([128, 2, D], BF16)
        tri_t = const.tile([128, 1024], F32)
        ones512 = const.tile([1, 512], BF16)
        ones64f = const.tile([1, 64], F32)
        bq_t = const.tile([128, 2], F32)
        bk_t = const.tile([128, 2], F32)
        bv_t = const.tile([1, CHA], BF16)

        sm = ctx.enter_context(tc.tile_pool(name="sm", bufs=4))      # pt tiles
        recp = ctx.enter_context(tc.tile_pool(name="recp", bufs=2))  # 1/den rows
        bcsp = ctx.enter_context(tc.tile_pool(name="bcsp", bufs=2))  # bcast 1/den
        sop = ctx.enter_context(tc.tile_pool(name="sop", bufs=3))    # out stage
        # PSUM budget: 1 + 4 + 3 = 8 banks exactly
        ps512 = ctx.enter_context(
            tc.tile_pool(name="ps512", bufs=1, space="PSUM"))
        stp = ctx.enter_context(tc.tile_pool(name="stp", bufs=2, space="PSUM"))
        pvp = ctx.enter_context(tc.tile_pool(name="pvp", bufs=3, space="PSUM"))

        # ---- DMA. Descriptor economics dominate: each descriptor costs
        # ~90ns on its queue and covers one partition-row run, so (a) weights
        # are host-permuted so 8 chunk-rows per partition are DRAM-contiguous
        # (2KB+ descriptors), (b) startup-critical transfers are split by
        # partition halves to run on more queues, (c) issue is spread across
        # three sequencers (~620ns per dma_start per sequencer).
        xTr = xT.rearrange("(a p) s -> a p s", p=128)
        wqp = wq.rearrange("(p c) ch -> p c ch", c=8)
        wkp = wk.rearrange("(p c) ch -> p c ch", c=8)
        wvp = wv.rearrange("(p c) ch -> p c ch", c=8)
        wop = wo.rearrange("(p c) n -> p c n", c=2)
        nc.sync.dma_start(ones64f, onesf)
        nc.sync.dma_start(bq_t, bq)
        nc.sync.dma_start(bk_t, bk)
        nc.scalar.dma_start(bv_t, bv)
        nc.scalar.dma_start(ones512, ones)
        issuers = [nc.sync, nc.scalar, nc.gpsimd]
        it = 0

        def issue(dst, src):
            nonlocal it
            issuers[it % 3].dma_start(dst, src)
            it += 1

        # startup-critical: x s0:512 (all c) then the Q/K/V weights
        for c in range(8):
            for ph in range(2):
                psl = slice(ph * 64, (ph + 1) * 64)
                issue(xt[psl, c, 0:512], xTr[c][psl, 0:512])
        for w_t, wsrc in ((wq_t, wqp), (wk_t, wkp), (wv_t, wvp)):
            for cq in range(2):
                csl = slice(cq * 4, (cq + 1) * 4)
                for ph in range(2):
                    psl = slice(ph * 64, (ph + 1) * 64)
                    issue(w_t[psl, csl, :], wsrc[psl, csl, :])
        # staircase mask (only cols 128:640 are ever read)
        for ph in range(2):
            psl = slice(ph * 64, (ph + 1) * 64)
            issue(tri_t[psl, 128:640], tri[psl, :])
        # rest of x
        for c in range(8):
            for ph in range(2):
                psl = slice(ph * 64, (ph + 1) * 64)
                issue(xt[psl, c, 512:S], xTr[c][psl, 512:S])
        for ph in range(2):
            psl = slice(ph * 64, (ph + 1) * 64)
            nc.sync.dma_start(wo_t[psl, :, :], wop[psl, :, :])

        # preload the ACT table while ACT is otherwise idle
        nc.scalar.activation(ones64f, ones64f, AF.Ln)

        # ---- work-item emitters -----------------------------------------
        def emit_qk_tile(which, m, n, pool=None):
            w_t, dst, bias = ((wq_t, qt, bq_t) if which == "q"
                              else (wk_t, kt, bk_t))
            ps = (pool or ps512).tile([128, 512], F32, tag="pv" if pool
                                      else "ps", name="ps")
            for c in range(8):
                nc.tensor.matmul(ps, w_t[:, c, m * 128:(m + 1) * 128],
                                 xt[:, c, n * 512:(n + 1) * 512],
                                 start=(c == 0), stop=(c == 7))
            dstv = dst[:, m, n * 512:(n + 1) * 512]
            # bias folded into the evacuation as a per-partition scalar
            # (host pre-scales bq by 0.125)
            nc.vector.tensor_scalar_add(dstv, ps, bias[:, m:m + 1])

        def emit_v_tile(t, pool=None):
            ps = (pool or ps512).tile([128, 512], F32, tag="pv" if pool
                                      else "ps", name="ps")
            psv = ps[:, 0:CHA]
            for c in range(8):
                nc.tensor.matmul(psv, xt[:, c, t * 128:(t + 1) * 128],
                                 wv_t[:, c, :], start=(c == 0), stop=False)
            # bias row (contains the 1.0 for the ones columns)
            nc.tensor.matmul(psv, ones512[0:1, 0:128], bv_t, start=False,
                             stop=True)
            nc.vector.tensor_copy(va[:, t, :], psv)

        so_map = {}

        def emit_outproj_tile(t, n):
            ps = ps512.tile([128, 512], F32, tag="ps", name="ps")
            for c2 in range(2):
                nc.tensor.matmul(ps, otn[:, c2, t * 128:(t + 1) * 128],
                                 wo_t[:, c2, n * 512:(n + 1) * 512],
                                 start=(c2 == 0), stop=(c2 == 1))
            if t not in so_map:
                so_map[t] = sop.tile([128, 1024], BF16, tag="so", name="so")
            so = so_map[t]
            nc.vector.tensor_copy(so[:, n * 512:(n + 1) * 512], ps)
            if n == 1:
                # full-D bf16 rows -> one 2KB descriptor per partition
                for ph in range(2):
                    psl = slice(ph * 64, (ph + 1) * 64)
                    orows = slice(t * 128 + ph * 64, t * 128 + (ph + 1) * 64)
                    nc.gpsimd.dma_start(out[orows, :], so[psl, :])
                del so_map[t]

        # ---- filler queue ------------------------------------------------
        queue = deque()

        def pop_filler(k=1):
            for _ in range(k):
                if not queue:
                    return
                _, fn = queue.popleft()
                fn()

        def flush_required(keys):
            while queue and any(k in keys for k, _ in queue):
                _, fn = queue.popleft()
                fn()

        def mk(fn, *a):
            return lambda: fn(*a)

        # needs of section (j,p), in consumption order; prologue covers
        # (0,0): Qm0n0, Km0n0, V0-3.
        for j in range(NQ):
            for p in range(2):
                if j == 0 and p == 0:
                    continue
                if p == 0:
                    queue.append((("k", 0, j), mk(emit_qk_tile, "k", 0, j)))
                    for t in range(4 * j, 4 * j + 4):
                        queue.append((("v", t), mk(emit_v_tile, t)))
                    queue.append((("q", 0, j), mk(emit_qk_tile, "q", 0, j)))
                else:
                    queue.append((("q", 1, j), mk(emit_qk_tile, "q", 1, j)))
                    queue.append((("k", 1, j), mk(emit_qk_tile, "k", 1, j)))

        def section_needs(j, p):
            keys = {("q", p, j)}
            for n in range(j + 1):
                keys.add(("k", p, n))
            for t in range(4 * (j + 1)):
                keys.add(("v", t))
            return keys

        # ---- prologue: minimal projections for attention (0,0) ----------
        # (runs in the pvp pool: attention hasn't claimed those banks yet)
        emit_qk_tile("q", 0, 0, pool=pvp)
        emit_qk_tile("k", 0, 0, pool=pvp)
        for t in range(4):
            emit_v_tile(t, pool=pvp)

        # ---- attention backbone -----------------------------------------
        for j in range(NQ):
            nkb = 4 * (j + 1)
            qsl = slice(j * 512, (j + 1) * 512)
            for p in range(2):
                flush_required(section_needs(j, p))
                pv = [pvp.tile([128, 512], F32, tag="pv", name=f"pv{hh}")
                      for hh in range(2)]

                def emit_pv(pt_, kb_):
                    for hh in range(2):
                        h = 2 * p + hh
                        nc.tensor.matmul(
                            pv[hh][0:65, :], va[:, kb_, h * 65:h * 65 + 65],
                            pt_[:, hh * 512:(hh + 1) * 512],
                            start=(kb_ == 0), stop=(kb_ == nkb - 1),
                            skip_group_check=True)

                pending = deque()
                for kb in range(nkb):
                    st = stp.tile([128, 1024], F32, tag="st", name="st")
                    for hh in range(2):
                        oh = hh * 64
                        nc.tensor.matmul(
                            st[:, hh * 512:(hh + 1) * 512],
                            kt[oh:oh + 64, p, kb * 128:(kb + 1) * 128],
                            qt[oh:oh + 64, p, qsl], start=True, stop=True)
                    rel = kb * 128 - j * 512
                    if rel >= 0:
                        # causal staircase bias over cols [0, rel+128)
                        for hh in range(2):
                            sl = st[:, hh * 512:hh * 512 + rel + 128]
                            nc.vector.tensor_add(sl, sl,
                                                 tri_t[:, 512 - rel:640])
                    pt = sm.tile([128, 1024], BF16, tag="pt", name="pt")
                    nc.scalar.activation(pt, st, AF.Exp)
                    # defer filler work to the late, exp-paced sections so
                    # the PE never idles long enough to re-throttle (HAM)
                    pop_filler(0 if j <= 1 else (1 if j == 2 else 2))
                    if len(pending) == 2:
                        emit_pv(*pending.popleft())
                    pending.append((pt, kb))
                while pending:
                    pop_filler()
                    emit_pv(*pending.popleft())
                # normalize per head: 1/den = exp(-ln(den)) on ACT with Ln
                # reading the denominator row straight from PSUM, partition-
                # broadcast on GPSIMD (idle engine), multiply on DVE. Short
                # per-hh chains so the pv pool slot frees ASAP.
                for hh in range(2):
                    oh = hh * 64
                    den = recp.tile([1, 512], F32, tag="rec", name="den")
                    nc.scalar.activation(den, pv[hh][64:65, :], AF.Ln)
                    nc.scalar.activation(den, den, AF.Exp, scale=-1.0)
                    bcs = bcsp.tile([64, 512], F32, tag="bcs", name="bcs")
                    nc.gpsimd.partition_broadcast(bcs, den, channels=64)
                    nc.vector.tensor_mul(otn[oh:oh + 64, p, qsl],
                                         pv[hh][0:64, :], bcs)
            # output projection for this q-block becomes filler work
            for t in range(4 * j, 4 * j + 4):
                for n in range(2):
                    queue.append((("o", t, n), mk(emit_outproj_tile, t, n)))
        # drain remaining fillers (late out-projs)
        while queue:
            _, fn = queue.popleft()
            fn()

    nc.compile()
    return nc


def _tri_np():
    # staircase causal bias: tri[kk, x] = NEG if x < 512+kk else 0, cols
    # 128:640 of the logical [128,1024] table (the only columns ever read)
    xs = np.arange(128, 640)[None, :]
    ks = np.arange(128)[:, None]
    return np.where(xs < 512 + ks, np.float32(NEG),
                    np.float32(0.0)).astype(np.float32)


def _perm2_wo(w):
    # wo rows permuted so a partition's 2 chunk-rows are DRAM-contiguous
    ch, d = w.shape
    return np.ascontiguousarray(
        w.reshape(2, ch // 2, d).transpose(1, 0, 2).reshape(ch, d))


def _perm8(w):
    # row r of the permuted layout = row (r%8)*128 + r//8 of w, so the 8
    # contraction-chunk rows a partition needs are DRAM-contiguous
    d, ch = w.shape
    return np.ascontiguousarray(
        w.reshape(8, d // 8, ch).transpose(1, 0, 2).reshape(d, ch))


def build_in_maps(x, Wq, bq, Wk, bk, Wv, bv, Wo):
    tri_np = _tri_np()
    ones_np = np.ones((1, 512), dtype=np.float32)
    xT_b = [np.ascontiguousarray(x[b].T) for b in range(B)]
    in_maps = []
    for c in range(N_CORES):
        b, tp = divmod(c, TPG)
        sl = slice(tp * CH, (tp + 1) * CH)
        wv_aug = np.zeros((D, CHA), dtype=np.float32)
        bv_aug = np.zeros((1, CHA), dtype=np.float32)
        for h in range(HPC):
            hsl = slice(tp * CH + h * DH, tp * CH + (h + 1) * DH)
            wv_aug[:, h * 65:h * 65 + DH] = Wv[:, hsl]
            bv_aug[0, h * 65:h * 65 + DH] = bv[hsl]
            bv_aug[0, h * 65 + DH] = 1.0
        in_maps.append({
            "xT": xT_b[b].astype(ml_dtypes.bfloat16),
            "wq": _perm8((Wq[:, sl].astype(np.float32)
                          * 0.125).astype(ml_dtypes.bfloat16)),
            "wk": _perm8(
                Wk[:, sl].astype(np.float32).astype(ml_dtypes.bfloat16)),
            "wv": _perm8(wv_aug.astype(ml_dtypes.bfloat16)),
            "wo": _perm2_wo(
                Wo[sl, :].astype(np.float32).astype(ml_dtypes.bfloat16)),
            "bq": (bq[sl].astype(np.float32) * 0.125).reshape(2, 128).T.copy(),
            "bk": bk[sl].astype(np.float32).reshape(2, 128).T.copy(),
            "bv": bv_aug.astype(ml_dtypes.bfloat16),
            "tri": tri_np,
            "ones": ones_np.astype(ml_dtypes.bfloat16),
            "onesf": ones_np[:, :64].copy(),
        })
    return in_maps


def _get_program():
    global _PROG
    if _PROG is None:
        _PROG = _build_program()
    return _PROG


def kernel(x, mask, Wq, bq, Wk, bk, Wv, bv, Wo, bo):
    x = np.asarray(x, dtype=np.float32)
    mask = np.asarray(mask)
    Wq, Wk, Wv, Wo = (np.asarray(w, dtype=np.float32)
                      for w in (Wq, Wk, Wv, Wo))
    bq, bk, bv, bo = (np.asarray(b, dtype=np.float32)
                      for b in (bq, bk, bv, bo))
    causal = bool(
        np.array_equal(mask != 0,
                       np.tril(np.ones((S, S), dtype=bool))))
    if not causal:
        # Fallback for non-causal masks: exact host computation.
        q = (x @ Wq + bq).reshape(B, S, H, DH).transpose(0, 2, 1, 3)
        k = (x @ Wk + bk).reshape(B, S, H, DH).transpose(0, 2, 1, 3)
        v = (x @ Wv + bv).reshape(B, S, H, DH).transpose(0, 2, 1, 3)
        attn = np.einsum("bhqd,bhkd->bhqk", q, k) / np.sqrt(np.float32(DH))
        attn = np.where(mask == 0, np.float32(-1e9), attn)
        attn = attn - attn.max(axis=-1, keepdims=True)
        e = np.exp(attn)
        p = e / e.sum(axis=-1, keepdims=True)
        o = np.einsum("bhqk,bhkd->bhqd", p, v)
        o = o.transpose(0, 2, 1, 3).reshape(B, S, D)
        return (o @ Wo + bo).astype(np.float32)

    nc = _get_program()
    in_maps = build_in_maps(x, Wq, bq, Wk, bk, Wv, bv, Wo)
    res = run_bass_kernel_spmd(nc, in_maps, core_ids=list(range(N_CORES)))
    out = np.zeros((B, S, D), dtype=np.float32)
    for c in range(N_CORES):
        out[c // TPG] += res.results[c]["out"].astype(np.float32)
    out += bo.astype(np.float32)
    return out


# revision 16
# speedup vs baseline: 1.0016x; 1.0016x over previous
"""Multi-head causal self-attention (B=2, S=2048, D=1024, H=16) on 8 TRN2 NeuronCores.

Sharding: data-parallel over batch (2) x tensor-parallel over heads (4 groups of
4 heads). Each core computes Q/K/V projections for its 4 heads, causal
flash-style attention (scores kept transposed [k, q] so no on-chip transposes
are needed), and a partial output projection against its row-slice of W_O.
Host sums the 4 partials per batch and adds the output bias.

v2: single software-pipelined program. Attention kb-steps (QK pair -> staircase
add -> exp -> PV pair) form the backbone; projection tiles and output-projection
tiles are interleaved into the PE stream as filler so the PE never idles (keeps
the HAM clock gate at full rate). Softmax denominators come from an all-ones
column appended to V; 1/den = exp(-ln(den)) on ACT (single pinned activation
table: natural_log_exp_and_others), broadcast across partitions on the idle
GPSIMD engine (no DRAM bounce), multiplied on DVE. Q/K biases are folded into
the PSUM evacuations as per-partition scalars. DMA issues are spread across
four engine sequencers so descriptor-generation latency doesn't gate startup.
"""

import contextlib
import sys
from collections import deque

import ml_dtypes
import numpy as np

sys.path.insert(0, "/opt/trn_rl_repo")

import bass_rust as _bass_rust  # noqa: E402
import concourse.bass as bass  # noqa: E402,F401
import concourse.tile as tile  # noqa: E402
from concourse import bacc, mybir  # noqa: E402
from concourse.bass_utils import run_bass_kernel_spmd  # noqa: E402
from concourse.hw_specs import get_activation_tables  # noqa: E402

F32 = mybir.dt.float32
F32R = mybir.dt.float32r
BF16 = mybir.dt.bfloat16
AF = mybir.ActivationFunctionType

B, S, D, H = 2, 2048, 1024, 16
DH = D // H          # 64
TPG = 4              # tensor-parallel groups
HPC = H // TPG       # 4 heads per core
CH = HPC * DH        # 256 channels per core
CHA = CH + HPC       # 260: V channels augmented with a ones column per head
NEG = -1.0e9
N_CORES = 8
NQ = S // 512        # 4 q-blocks of 512
NT = S // 128        # 16 s-tiles / k-blocks

ACT_TABLE = "natural_log_exp_and_others"  # ln + exp + copy in one table

_PROG = None  # cached compiled Bass program


def _pin_act_table(nc):
    """Constrain the activation-table chooser to ACT_TABLE so ln/exp/copy
    never thrash between table sets (each reload is 1283ns on the ACT
    engine, and lands on the softmax critical path). Table ids stay index-
    aligned with act_info.json; non-target sets are emptied so first-match
    lands on the combined table."""
    tables = [
        (name, (funcs if name == ACT_TABLE else set()))
        for name, funcs in get_activation_tables(nc.m.arch).items()
    ]
    assert any(name == ACT_TABLE and funcs for name, funcs in tables)
    nc.insert_act_table_loads = (
        lambda: _bass_rust.insert_act_table_loads(nc, tables))


def _build_program():
    nc = bacc.Bacc("TRN2", target_bir_lowering=False, debug=False,
                   num_devices=N_CORES)
    _pin_act_table(nc)

    xT = nc.dram_tensor("xT", [D, S], BF16, kind="ExternalInput").ap()
    wq = nc.dram_tensor("wq", [D, CH], BF16, kind="ExternalInput").ap()
    wk = nc.dram_tensor("wk", [D, CH], BF16, kind="ExternalInput").ap()
    wv = nc.dram_tensor("wv", [D, CHA], BF16, kind="ExternalInput").ap()
    wo = nc.dram_tensor("wo", [CH, D], BF16, kind="ExternalInput").ap()
    bq = nc.dram_tensor("bq", [128, 2], F32, kind="ExternalInput").ap()
    bk = nc.dram_tensor("bk", [128, 2], F32, kind="ExternalInput").ap()
    bv = nc.dram_tensor("bv", [1, CHA], BF16, kind="ExternalInput").ap()
    tri = nc.dram_tensor("tri", [128, 512], F32, kind="ExternalInput").ap()
    ones = nc.dram_tensor("ones", [1, 512], BF16, kind="ExternalInput").ap()
    onesf = nc.dram_tensor("onesf", [1, 64], F32, kind="ExternalInput").ap()
    out = nc.dram_tensor("out", [S, D], BF16, kind="ExternalOutput").ap()

    with tile.TileContext(nc) as tc, contextlib.ExitStack() as ctx:
        const = ctx.enter_context(tc.tile_pool(name="const", bufs=1))
        xt = const.tile([128, 8, S], BF16)
        wq_t = const.tile([128, 8, CH], BF16)
        wk_t = const.tile([128, 8, CH], BF16)
        wv_t = const.tile([128, 8, CHA], BF16)
        qt = const.tile([128, 2, S], F32R)     # Q^T/8 (+bq/8): rows 0-63 even head
        kt = const.tile([128, 2, S], F32R)     # K^T (+bk)
        va = const.tile([128, NT, CHA], BF16)  # V augmented, head-major 65-col blocks
        otn = const.tile([128, 2, S], BF16)    # normalized attention out, transposed
        wo_t = const.tile([128, 2, D], BF16)
        tri_t = const.tile([128, 1024], F32)
        ones512 = const.tile([1, 512], BF16)
        ones64f = const.tile([1, 64], F32)
        bq_t = const.tile([128, 2], F32)
        bk_t = const.tile([128, 2], F32)
        bv_t = const.tile([1, CHA], BF16)

        sm = ctx.enter_context(tc.tile_pool(name="sm", bufs=4))      # pt tiles
        recp = ctx.enter_context(tc.tile_pool(name="recp", bufs=2))  # 1/den rows
        bcsp = ctx.enter_context(tc.tile_pool(name="bcsp", bufs=2))  # bcast 1/den
        sop = ctx.enter_context(tc.tile_pool(name="sop", bufs=3))    # out stage
        # PSUM budget: 1 + 4 + 3 = 8 banks exactly
        ps512 = ctx.enter_context(
            tc.tile_pool(name="ps512", bufs=1, space="PSUM"))
        stp = ctx.enter_context(tc.tile_pool(name="stp", bufs=2, space="PSUM"))
        pvp = ctx.enter_context(tc.tile_pool(name="pvp", bufs=3, space="PSUM"))

        # ---- DMA. Descriptor economics dominate: each descriptor costs
        # ~90ns on its queue and covers one partition-row run, so (a) weights
        # are host-permuted so 8 chunk-rows per partition are DRAM-contiguous
        # (2KB+ descriptors), (b) startup-critical transfers are split by
        # partition halves to run on more queues, (c) issue is spread across
        # three sequencers (~620ns per dma_start per sequencer).
        xTr = xT.rearrange("(a p) s -> a p s", p=128)
        wqp = wq.rearrange("(p c) ch -> p c ch", c=8)
        wkp = wk.rearrange("(p c) ch -> p c ch", c=8)
        wvp = wv.rearrange("(p c) ch -> p c ch", c=8)
        wop = wo.rearrange("(p c) n -> p c n", c=2)
        nc.sync.dma_start(ones64f, onesf)
        nc.sync.dma_start(bq_t, bq)
        nc.sync.dma_start(bk_t, bk)
        nc.scalar.dma_start(bv_t, bv)
        nc.scalar.dma_start(ones512, ones)
        issuers = [nc.sync, nc.scalar, nc.gpsimd]
        it = 0

        def issue(dst, src):
            nonlocal it
            issuers[it % 3].dma_start(dst, src)
            it += 1

        # startup-critical: xt c0 + wq first (the first Q-proj matmuls),
        # then wk, the rest of x s0:512, wv
        for ph in range(2):
            psl = slice(ph * 64, (ph + 1) * 64)
            issue(xt[psl, 0, 0:512], xTr[0][psl, 0:512])
        for w_t, wsrc in ((wq_t, wqp), (wk_t, wkp)):
            for cq in range(2):
                csl = slice(cq * 4, (cq + 1) * 4)
                for ph in range(2):
                    psl = slice(ph * 64, (ph + 1) * 64)
                    issue(w_t[psl, csl, :], wsrc[psl, csl, :])
        for c in range(1, 8):
            for ph in range(2):
                psl = slice(ph * 64, (ph + 1) * 64)
                issue(xt[psl, c, 0:512], xTr[c][psl, 0:512])
        for cq in range(2):
            csl = slice(cq * 4, (cq + 1) * 4)
            for ph in range(2):
                psl = slice(ph * 64, (ph + 1) * 64)
                issue(wv_t[psl, csl, :], wvp[psl, csl, :])
        # staircase mask (only cols 128:640 are ever read)
        for ph in range(2):
            psl = slice(ph * 64, (ph + 1) * 64)
            issue(tri_t[psl, 128:640], tri[psl, :])
        # rest of x
        for c in range(8):
            for ph in range(2):
                psl = slice(ph * 64, (ph + 1) * 64)
                issue(xt[psl, c, 512:S], xTr[c][psl, 512:S])
        for ph in range(2):
            psl = slice(ph * 64, (ph + 1) * 64)
            nc.sync.dma_start(wo_t[psl, :, :], wop[psl, :, :])

        # preload the ACT table while ACT is otherwise idle
        nc.scalar.activation(ones64f, ones64f, AF.Ln)

        # ---- work-item emitters -----------------------------------------
        def emit_qk_tile(which, m, n, pool=None):
            w_t, dst, bias = ((wq_t, qt, bq_t) if which == "q"
                              else (wk_t, kt, bk_t))
            ps = (pool or ps512).tile([128, 512], F32, tag="pv" if pool
                                      else "ps", name="ps")
            for c in range(8):
                nc.tensor.matmul(ps, w_t[:, c, m * 128:(m + 1) * 128],
                                 xt[:, c, n * 512:(n + 1) * 512],
                                 start=(c == 0), stop=(c == 7))
            dstv = dst[:, m, n * 512:(n + 1) * 512]
            # bias folded into the evacuation as a per-partition scalar
            # (host pre-scales bq by 0.125)
            nc.vector.tensor_scalar_add(dstv, ps, bias[:, m:m + 1])

        def emit_v_tile(t, pool=None):
            ps = (pool or ps512).tile([128, 512], F32, tag="pv" if pool
                                      else "ps", name="ps")
            psv = ps[:, 0:CHA]
            for c in range(8):
                nc.tensor.matmul(psv, xt[:, c, t * 128:(t + 1) * 128],
                                 wv_t[:, c, :], start=(c == 0), stop=False)
            # bias row (contains the 1.0 for the ones columns)
            nc.tensor.matmul(psv, ones512[0:1, 0:128], bv_t, start=False,
                             stop=True)
            nc.vector.tensor_copy(va[:, t, :], psv)

        so_map = {}

        def emit_outproj_tile(t, n, pool=None):
            ps = (pool or ps512).tile([128, 512], F32, tag="pv" if pool
                                      else "ps", name="ps")
            for c2 in range(2):
                nc.tensor.matmul(ps, otn[:, c2, t * 128:(t + 1) * 128],
                                 wo_t[:, c2, n * 512:(n + 1) * 512],
                                 start=(c2 == 0), stop=(c2 == 1))
            if t not in so_map:
                so_map[t] = sop.tile([128, 1024], BF16, tag="so", name="so")
            so = so_map[t]
            nc.vector.tensor_copy(so[:, n * 512:(n + 1) * 512], ps)
            if n == 1:
                # full-D bf16 rows -> one 2KB descriptor per partition; four
                # partition-quarter starts spread the descriptor processing
                for ph in range(4):
                    psl = slice(ph * 32, (ph + 1) * 32)
                    orows = slice(t * 128 + ph * 32, t * 128 + (ph + 1) * 32)
                    nc.gpsimd.dma_start(out[orows, :], so[psl, :])
                del so_map[t]

        # ---- filler queue ------------------------------------------------
        queue = deque()

        def pop_filler(k=1):
            for _ in range(k):
                if not queue:
                    return
                _, fn = queue.popleft()
                fn()

        def flush_required(keys):
            while queue and any(k in keys for k, _ in queue):
                _, fn = queue.popleft()
                fn()

        def mk(fn, *a):
            return lambda: fn(*a)

        # needs of section (j,p), in consumption order; prologue covers
        # (0,0): Qm0n0, Km0n0, V0-3.
        for j in range(NQ):
            for p in range(2):
                if j == 0 and p == 0:
                    continue
                if p == 0:
                    queue.append((("k", 0, j), mk(emit_qk_tile, "k", 0, j)))
                    for t in range(4 * j, 4 * j + 4):
                        queue.append((("v", t), mk(emit_v_tile, t)))
                    queue.append((("q", 0, j), mk(emit_qk_tile, "q", 0, j)))
                else:
                    queue.append((("q", 1, j), mk(emit_qk_tile, "q", 1, j)))
                    queue.append((("k", 1, j), mk(emit_qk_tile, "k", 1, j)))

        def section_needs(j, p):
            keys = {("q", p, j)}
            for n in range(j + 1):
                keys.add(("k", p, n))
            for t in range(4 * (j + 1)):
                keys.add(("v", t))
            return keys

        # ---- prologue: minimal projections for attention (0,0) ----------
        # (runs in the pvp pool: attention hasn't claimed those banks yet)
        emit_qk_tile("q", 0, 0, pool=pvp)
        emit_qk_tile("k", 0, 0, pool=pvp)
        for t in range(4):
            emit_v_tile(t, pool=pvp)

        # ---- attention backbone -----------------------------------------
        for j in range(NQ):
            nkb = 4 * (j + 1)
            qsl = slice(j * 512, (j + 1) * 512)
            for p in range(2):
                flush_required(section_needs(j, p))
                pv = [pvp.tile([128, 512], F32, tag="pv", name=f"pv{hh}")
                      for hh in range(2)]

                def emit_pv(pt_, kb_):
                    for hh in range(2):
                        h = 2 * p + hh
                        nc.tensor.matmul(
                            pv[hh][0:65, :], va[:, kb_, h * 65:h * 65 + 65],
                            pt_[:, hh * 512:(hh + 1) * 512],
                            start=(kb_ == 0), stop=(kb_ == nkb - 1),
                            skip_group_check=True)

                pending = deque()
                for kb in range(nkb):
                    st = stp.tile([128, 1024], F32, tag="st", name="st")
                    for hh in range(2):
                        oh = hh * 64
                        nc.tensor.matmul(
                            st[:, hh * 512:(hh + 1) * 512],
                            kt[oh:oh + 64, p, kb * 128:(kb + 1) * 128],
                            qt[oh:oh + 64, p, qsl], start=True, stop=True)
                    rel = kb * 128 - j * 512
                    if rel >= 0:
                        # causal staircase bias over cols [0, rel+128)
                        for hh in range(2):
                            sl = st[:, hh * 512:hh * 512 + rel + 128]
                            nc.vector.tensor_add(sl, sl,
                                                 tri_t[:, 512 - rel:640])
                    pt = sm.tile([128, 1024], BF16, tag="pt", name="pt")
                    nc.scalar.activation(pt, st, AF.Exp)
                    # defer filler work to the late, exp-paced sections so
                    # the PE never idles long enough to re-throttle (HAM)
                    if j == 2:
                        pop_filler(1)
                    elif j == 3:
                        pop_filler(1 if (p == 0 and kb % 2 == 0) else
                                   (1 if p == 1 else 0))
                    if len(pending) == 2:
                        emit_pv(*pending.popleft())
                    pending.append((pt, kb))
                while pending:
                    pop_filler()
                    emit_pv(*pending.popleft())
                # normalize per head: 1/den = exp(-ln(den)) on ACT with Ln
                # reading the denominator row straight from PSUM, partition-
                # broadcast on GPSIMD (idle engine), multiply on DVE. Short
                # per-hh chains so the pv pool slot frees ASAP.
                for hh in range(2):
                    oh = hh * 64
                    den = recp.tile([1, 512], F32, tag="rec", name="den")
                    nc.scalar.activation(den, pv[hh][64:65, :], AF.Ln)
                    nc.scalar.activation(den, den, AF.Exp, scale=-1.0)
                    bcs = bcsp.tile([64, 512], F32, tag="bcs", name="bcs")
                    nc.gpsimd.partition_broadcast(bcs, den, channels=64)
                    nc.vector.tensor_mul(otn[oh:oh + 64, p, qsl],
                                         pv[hh][0:64, :], bcs)
            # output projection for this q-block becomes filler work
            for t in range(4 * j, 4 * j + 4):
                for n in range(2):
                    queue.append((("o", t, n), mk(emit_outproj_tile, t, n)))
        # drain remaining fillers (late out-projs); attention is done, so
        # alternate between the ps512 and freed pvp banks to pipeline the
        # matmul/evacuate/DMA chain
        drain_i = 0
        while queue:
            key, fn = queue.popleft()
            if key[0] == "o" and drain_i % 2 == 1:
                emit_outproj_tile(key[1], key[2], pool=pvp)
            else:
                fn()
            drain_i += 1

    nc.compile()
    return nc


def _tri_np():
    # staircase causal bias: tri[kk, x] = NEG if x < 512+kk else 0, cols
    # 128:640 of the logical [128,1024] table (the only columns ever read)
    xs = np.arange(128, 640)[None, :]
    ks = np.arange(128)[:, None]
    return np.where(xs < 512 + ks, np.float32(NEG),
                    np.float32(0.0)).astype(np.float32)


def _perm2_wo(w):
    # wo rows permuted so a partition's 2 chunk-rows are DRAM-contiguous
    ch, d = w.shape
    return np.ascontiguousarray(
        w.reshape(2, ch // 2, d).transpose(1, 0, 2).reshape(ch, d))


def _perm8(w):
    # row r of the permuted layout = row (r%8)*128 + r//8 of w, so the 8
    # contraction-chunk rows a partition needs are DRAM-contiguous
    d, ch = w.shape
    return np.ascontiguousarray(
        w.reshape(8, d // 8, ch).transpose(1, 0, 2).reshape(d, ch))


def build_in_maps(x, Wq, bq, Wk, bk, Wv, bv, Wo):
    tri_np = _tri_np()
    ones_np = np.ones((1, 512), dtype=np.float32)
    xT_b = [np.ascontiguousarray(x[b].T) for b in range(B)]
    in_maps = []
    for c in range(N_CORES):
        b, tp = divmod(c, TPG)
        sl = slice(tp * CH, (tp + 1) * CH)
        wv_aug = np.zeros((D, CHA), dtype=np.float32)
        bv_aug = np.zeros((1, CHA), dtype=np.float32)
        for h in range(HPC):
            hsl = slice(tp * CH + h * DH, tp * CH + (h + 1) * DH)
            wv_aug[:, h * 65:h * 65 + DH] = Wv[:, hsl]
            bv_aug[0, h * 65:h * 65 + DH] = bv[hsl]
            bv_aug[0, h * 65 + DH] = 1.0
        in_maps.append({
            "xT": xT_b[b].astype(ml_dtypes.bfloat16),
            "wq": _perm8((Wq[:, sl].astype(np.float32)
                          * 0.125).astype(ml_dtypes.bfloat16)),
            "wk": _perm8(
                Wk[:, sl].astype(np.float32).astype(ml_dtypes.bfloat16)),
            "wv": _perm8(wv_aug.astype(ml_dtypes.bfloat16)),
            "wo": _perm2_wo(
                Wo[sl, :].astype(np.float32).astype(ml_dtypes.bfloat16)),
            "bq": (bq[sl].astype(np.float32) * 0.125).reshape(2, 128).T.copy(),
            "bk": bk[sl].astype(np.float32).reshape(2, 128).T.copy(),
            "bv": bv_aug.astype(ml_dtypes.bfloat16),
            "tri": tri_np,
            "ones": ones_np.astype(ml_dtypes.bfloat16),
            "onesf": ones_np[:, :64].copy(),
        })
    return in_maps


def _get_program():
    global _PROG
    if _PROG is None:
        _PROG = _build_program()
    return _PROG


def kernel(x, mask, Wq, bq, Wk, bk, Wv, bv, Wo, bo):
    x = np.asarray(x, dtype=np.float32)
    mask = np.asarray(mask)
    Wq, Wk, Wv, Wo = (np.asarray(w, dtype=np.float32)
                      for w in (Wq, Wk, Wv, Wo))
    bq, bk, bv, bo = (np.asarray(b, dtype=np.float32)
                      for b in (bq, bk, bv, bo))
    causal = bool(
        np.array_equal(mask != 0,
                       np.tril(np.ones((S, S), dtype=bool))))
    if not causal:
        # Fallback for non-causal masks: exact host computation.
        q = (x @ Wq + bq).reshape(B, S, H, DH).transpose(0, 2, 1, 3)
        k = (x @ Wk + bk).reshape(B, S, H, DH).transpose(0, 2, 1, 3)
        v = (x @ Wv + bv).reshape(B, S, H, DH).transpose(0, 2, 1, 3)
        attn = np.einsum("bhqd,bhkd->bhqk", q, k) / np.sqrt(np.float32(DH))
        attn = np.where(mask == 0, np.float32(-1e9), attn)
        attn = attn - attn.max(axis=-1, keepdims=True)
        e = np.exp(attn)
        p = e / e.sum(axis=-1, keepdims=True)
        o = np.einsum("bhqk,bhkd->bhqd", p, v)
        o = o.transpose(0, 2, 1, 3).reshape(B, S, D)
        return (o @ Wo + bo).astype(np.float32)

    nc = _get_program()
    in_maps = build_in_maps(x, Wq, bq, Wk, bk, Wv, bv, Wo)
    res = run_bass_kernel_spmd(nc, in_maps, core_ids=list(range(N_CORES)))
    out = np.zeros((B, S, D), dtype=np.float32)
    for c in range(N_CORES):
        out[c // TPG] += res.results[c]["out"].astype(np.float32)
    out += bo.astype(np.float32)
    return out


# revision 17
# speedup vs baseline: 1.0730x; 1.0712x over previous
"""Multi-head causal self-attention (B=2, S=2048, D=1024, H=16) on 8 TRN2 NeuronCores.

Sharding: data-parallel over batch (2) x tensor-parallel over heads (4 groups of
4 heads). Each core computes Q/K/V projections for its 4 heads, causal
flash-style attention (scores kept transposed [k, q] so no on-chip transposes
are needed), and a partial output projection against its row-slice of W_O.
Host sums the 4 partials per batch and adds the output bias.

v2: single software-pipelined program. Attention kb-steps (QK pair -> staircase
add -> exp -> PV pair) form the backbone; projection tiles and output-projection
tiles are interleaved into the PE stream as filler so the PE never idles (keeps
the HAM clock gate at full rate). Softmax denominators come from an all-ones
column appended to V; 1/den = exp(-ln(den)) on ACT (single pinned activation
table: natural_log_exp_and_others), broadcast across partitions on the idle
GPSIMD engine (no DRAM bounce), multiplied on DVE. Q/K biases are folded into
the PSUM evacuations as per-partition scalars. DMA issues are spread across
four engine sequencers so descriptor-generation latency doesn't gate startup.
"""

import contextlib
import sys
from collections import deque

import ml_dtypes
import numpy as np

sys.path.insert(0, "/opt/trn_rl_repo")

import bass_rust as _bass_rust  # noqa: E402
import concourse.bass as bass  # noqa: E402,F401
import concourse.tile as tile  # noqa: E402
from concourse import bacc, mybir  # noqa: E402
from concourse.bass_utils import run_bass_kernel_spmd  # noqa: E402
from concourse.hw_specs import get_activation_tables  # noqa: E402

F32 = mybir.dt.float32
F32R = mybir.dt.float32r
BF16 = mybir.dt.bfloat16
AF = mybir.ActivationFunctionType

B, S, D, H = 2, 2048, 1024, 16
DH = D // H          # 64
TPG = 4              # tensor-parallel groups
HPC = H // TPG       # 4 heads per core
CH = HPC * DH        # 256 channels per core
CHA = CH + HPC       # 260: V channels augmented with a ones column per head
NEG = -1.0e9
N_CORES = 8
NQ = S // 512        # 4 q-blocks of 512
NT = S // 128        # 16 s-tiles / k-blocks

ACT_TABLE = "natural_log_exp_and_others"  # ln + exp + copy in one table

_PROG = None  # cached compiled Bass program


def _pin_act_table(nc):
    """Constrain the activation-table chooser to ACT_TABLE so ln/exp/copy
    never thrash between table sets (each reload is 1283ns on the ACT
    engine, and lands on the softmax critical path). Table ids stay index-
    aligned with act_info.json; non-target sets are emptied so first-match
    lands on the combined table."""
    tables = [
        (name, (funcs if name == ACT_TABLE else set()))
        for name, funcs in get_activation_tables(nc.m.arch).items()
    ]
    assert any(name == ACT_TABLE and funcs for name, funcs in tables)
    nc.insert_act_table_loads = (
        lambda: _bass_rust.insert_act_table_loads(nc, tables))


def _build_program():
    nc = bacc.Bacc("TRN2", target_bir_lowering=False, debug=False,
                   num_devices=N_CORES)
    _pin_act_table(nc)

    xT = nc.dram_tensor("xT", [D, S], BF16, kind="ExternalInput").ap()
    wq = nc.dram_tensor("wq", [D, CH], BF16, kind="ExternalInput").ap()
    wk = nc.dram_tensor("wk", [D, CH], BF16, kind="ExternalInput").ap()
    wv = nc.dram_tensor("wv", [D, CHA], BF16, kind="ExternalInput").ap()
    wo = nc.dram_tensor("wo", [CH, D], BF16, kind="ExternalInput").ap()
    bq = nc.dram_tensor("bq", [128, 2], F32, kind="ExternalInput").ap()
    bk = nc.dram_tensor("bk", [128, 2], F32, kind="ExternalInput").ap()
    bv = nc.dram_tensor("bv", [1, CHA], BF16, kind="ExternalInput").ap()
    tri = nc.dram_tensor("tri", [128, 512], F32, kind="ExternalInput").ap()
    ones = nc.dram_tensor("ones", [1, 512], BF16, kind="ExternalInput").ap()
    onesf = nc.dram_tensor("onesf", [1, 64], F32, kind="ExternalInput").ap()
    out = nc.dram_tensor("out", [S, D], BF16, kind="ExternalOutput").ap()

    with tile.TileContext(nc) as tc, contextlib.ExitStack() as ctx:
        const = ctx.enter_context(tc.tile_pool(name="const", bufs=1))
        xt = const.tile([128, 8, S], BF16)
        wq_t = const.tile([128, 8, CH], BF16)
        wk_t = const.tile([128, 8, CH], BF16)
        wv_t = const.tile([128, 8, CHA], BF16)
        qt = const.tile([128, 2, S], F32R)     # Q^T/8 (+bq/8): rows 0-63 even head
        kt = const.tile([128, 2, S], F32R)     # K^T (+bk)
        va = const.tile([128, NT, CHA], BF16)  # V augmented, head-major 65-col blocks
        otn = const.tile([128, 2, S], BF16)    # normalized attention out, transposed
        wo_t = const.tile([128, 2, D], BF16)
        tri_t = const.tile([128, 1024], F32)
        ones512 = const.tile([1, 512], BF16)
        ones64f = const.tile([1, 64], F32)
        bq_t = const.tile([128, 2], F32)
        bk_t = const.tile([128, 2], F32)
        bv_t = const.tile([1, CHA], BF16)

        sm = ctx.enter_context(tc.tile_pool(name="sm", bufs=4))      # pt tiles
        recp = ctx.enter_context(tc.tile_pool(name="recp", bufs=2))  # 1/den rows
        bcsp = ctx.enter_context(tc.tile_pool(name="bcsp", bufs=2))  # bcast 1/den
        sop = ctx.enter_context(tc.tile_pool(name="sop", bufs=3))    # out stage
        # PSUM budget: 1 + 4 + 3 = 8 banks exactly
        ps512 = ctx.enter_context(
            tc.tile_pool(name="ps512", bufs=1, space="PSUM"))
        stp = ctx.enter_context(tc.tile_pool(name="stp", bufs=2, space="PSUM"))
        pvp = ctx.enter_context(tc.tile_pool(name="pvp", bufs=3, space="PSUM"))

        # ---- DMA. Descriptor economics dominate: each descriptor costs
        # ~90ns on its queue and covers one partition-row run, so (a) weights
        # are host-permuted so 8 chunk-rows per partition are DRAM-contiguous
        # (2KB+ descriptors), (b) startup-critical transfers are split by
        # partition halves to run on more queues, (c) issue is spread across
        # three sequencers (~620ns per dma_start per sequencer).
        xTr = xT.rearrange("(a p) s -> a p s", p=128)
        wqp = wq.rearrange("(p c) ch -> p c ch", c=8)
        wkp = wk.rearrange("(p c) ch -> p c ch", c=8)
        wvp = wv.rearrange("(p c) ch -> p c ch", c=8)
        wop = wo.rearrange("(p c) n -> p c n", c=2)
        nc.sync.dma_start(ones64f, onesf)
        nc.sync.dma_start(bq_t, bq)
        nc.sync.dma_start(bk_t, bk)
        nc.scalar.dma_start(bv_t, bv)
        nc.scalar.dma_start(ones512, ones)
        issuers = [nc.sync, nc.scalar, nc.gpsimd]
        it = 0

        def issue(dst, src):
            nonlocal it
            issuers[it % 3].dma_start(dst, src)
            it += 1

        # startup-critical: xt c0 + wq first (the first Q-proj matmuls),
        # then wk, the rest of x s0:512, wv
        for ph in range(2):
            psl = slice(ph * 64, (ph + 1) * 64)
            issue(xt[psl, 0, 0:512], xTr[0][psl, 0:512])
        for w_t, wsrc in ((wq_t, wqp), (wk_t, wkp)):
            for cq in range(2):
                csl = slice(cq * 4, (cq + 1) * 4)
                for ph in range(2):
                    psl = slice(ph * 64, (ph + 1) * 64)
                    issue(w_t[psl, csl, :], wsrc[psl, csl, :])
        for c in range(1, 8):
            for ph in range(2):
                psl = slice(ph * 64, (ph + 1) * 64)
                issue(xt[psl, c, 0:512], xTr[c][psl, 0:512])
        for cq in range(2):
            csl = slice(cq * 4, (cq + 1) * 4)
            for ph in range(2):
                psl = slice(ph * 64, (ph + 1) * 64)
                issue(wv_t[psl, csl, :], wvp[psl, csl, :])
        # staircase mask (only cols 128:640 are ever read)
        for ph in range(2):
            psl = slice(ph * 64, (ph + 1) * 64)
            issue(tri_t[psl, 128:640], tri[psl, :])
        # rest of x
        for c in range(8):
            for ph in range(2):
                psl = slice(ph * 64, (ph + 1) * 64)
                issue(xt[psl, c, 512:S], xTr[c][psl, 512:S])
        for ph in range(2):
            psl = slice(ph * 64, (ph + 1) * 64)
            nc.sync.dma_start(wo_t[psl, :, :], wop[psl, :, :])

        # preload the ACT table while ACT is otherwise idle
        nc.scalar.activation(ones64f, ones64f, AF.Ln)

        # ---- work-item emitters -----------------------------------------
        def emit_qk_tile(which, m, n, pool=None):
            w_t, dst, bias = ((wq_t, qt, bq_t) if which == "q"
                              else (wk_t, kt, bk_t))
            ps = (pool or ps512).tile([128, 512], F32, tag="pv" if pool
                                      else "ps", name="ps")
            for c in range(8):
                nc.tensor.matmul(ps, w_t[:, c, m * 128:(m + 1) * 128],
                                 xt[:, c, n * 512:(n + 1) * 512],
                                 start=(c == 0), stop=(c == 7))
            dstv = dst[:, m, n * 512:(n + 1) * 512]
            # bias folded into the evacuation as a per-partition scalar
            # (host pre-scales bq by 0.125)
            nc.vector.tensor_scalar_add(dstv, ps, bias[:, m:m + 1])

        def emit_v_tile(t, pool=None):
            ps = (pool or ps512).tile([128, 512], F32, tag="pv" if pool
                                      else "ps", name="ps")
            psv = ps[:, 0:CHA]
            for c in range(8):
                nc.tensor.matmul(psv, xt[:, c, t * 128:(t + 1) * 128],
                                 wv_t[:, c, :], start=(c == 0), stop=False)
            # bias row (contains the 1.0 for the ones columns)
            nc.tensor.matmul(psv, ones512[0:1, 0:128], bv_t, start=False,
                             stop=True)
            nc.vector.tensor_copy(va[:, t, :], psv)

        so_map = {}

        def emit_outproj_tile(t, n, pool=None):
            ps = (pool or ps512).tile([128, 512], F32, tag="pv" if pool
                                      else "ps", name="ps")
            for c2 in range(2):
                nc.tensor.matmul(ps, otn[:, c2, t * 128:(t + 1) * 128],
                                 wo_t[:, c2, n * 512:(n + 1) * 512],
                                 start=(c2 == 0), stop=(c2 == 1))
            if t not in so_map:
                so_map[t] = sop.tile([128, 1024], BF16, tag="so", name="so")
            so = so_map[t]
            nc.vector.tensor_copy(so[:, n * 512:(n + 1) * 512], ps)
            if n == 1:
                # full-D bf16 rows -> one 2KB descriptor per partition; two
                # partition-half starts, issue rotated across sequencers so
                # no single sequencer's ~620ns/issue rate gates the tail
                for ph in range(2):
                    psl = slice(ph * 64, (ph + 1) * 64)
                    orows = slice(t * 128 + ph * 64, t * 128 + (ph + 1) * 64)
                    issue(out[orows, :], so[psl, :])
                del so_map[t]

        # ---- filler queue ------------------------------------------------
        queue = deque()

        def pop_filler(k=1):
            for _ in range(k):
                if not queue:
                    return
                _, fn = queue.popleft()
                fn()

        def flush_required(keys):
            while queue and any(k in keys for k, _ in queue):
                _, fn = queue.popleft()
                fn()

        def mk(fn, *a):
            return lambda: fn(*a)

        # needs of section (j,p), in consumption order; prologue covers
        # (0,0): Qm0n0, Km0n0, V0-3.
        for j in range(NQ):
            for p in range(2):
                if j == 0 and p == 0:
                    continue
                if p == 0:
                    queue.append((("k", 0, j), mk(emit_qk_tile, "k", 0, j)))
                    for t in range(4 * j, 4 * j + 4):
                        queue.append((("v", t), mk(emit_v_tile, t)))
                    queue.append((("q", 0, j), mk(emit_qk_tile, "q", 0, j)))
                else:
                    queue.append((("q", 1, j), mk(emit_qk_tile, "q", 1, j)))
                    queue.append((("k", 1, j), mk(emit_qk_tile, "k", 1, j)))

        def section_needs(j, p):
            keys = {("q", p, j)}
            for n in range(j + 1):
                keys.add(("k", p, n))
            for t in range(4 * (j + 1)):
                keys.add(("v", t))
            return keys

        # ---- prologue: minimal projections for attention (0,0) ----------
        # (runs in the pvp pool: attention hasn't claimed those banks yet)
        emit_qk_tile("q", 0, 0, pool=pvp)
        emit_qk_tile("k", 0, 0, pool=pvp)
        for t in range(4):
            emit_v_tile(t, pool=pvp)

        # ---- attention backbone -----------------------------------------
        step_idx = [0]
        for j in range(NQ):
            nkb = 4 * (j + 1)
            qsl = slice(j * 512, (j + 1) * 512)
            for p in range(2):
                flush_required(section_needs(j, p))
                pv = [pvp.tile([128, 512], F32, tag="pv", name=f"pv{hh}")
                      for hh in range(2)]

                def emit_pv(pt_, kb_):
                    for hh in range(2):
                        h = 2 * p + hh
                        nc.tensor.matmul(
                            pv[hh][0:65, :], va[:, kb_, h * 65:h * 65 + 65],
                            pt_[:, hh * 512:(hh + 1) * 512],
                            start=(kb_ == 0), stop=(kb_ == nkb - 1),
                            skip_group_check=True)

                pending = deque()
                for kb in range(nkb):
                    st = stp.tile([128, 1024], F32, tag="st", name="st")
                    for hh in range(2):
                        oh = hh * 64
                        nc.tensor.matmul(
                            st[:, hh * 512:(hh + 1) * 512],
                            kt[oh:oh + 64, p, kb * 128:(kb + 1) * 128],
                            qt[oh:oh + 64, p, qsl], start=True, stop=True)
                    rel = kb * 128 - j * 512
                    if rel >= 0:
                        # causal staircase bias over cols [0, rel+128)
                        for hh in range(2):
                            sl = st[:, hh * 512:hh * 512 + rel + 128]
                            nc.vector.tensor_add(sl, sl,
                                                 tri_t[:, 512 - rel:640])
                    pt = sm.tile([128, 1024], BF16, tag="pt", name="pt")
                    nc.scalar.activation(pt, st, AF.Exp)
                    # defer filler work to the late, exp-paced sections so
                    # the PE never idles long enough to re-throttle (HAM)
                    step_idx[0] += 1
                    if step_idx[0] % 4 != 0:
                        pop_filler(1)
                    if len(pending) == 2:
                        emit_pv(*pending.popleft())
                    pending.append((pt, kb))
                while pending:
                    pop_filler()
                    emit_pv(*pending.popleft())
                # normalize per head: 1/den = exp(-ln(den)) on ACT with Ln
                # reading the denominator row straight from PSUM, partition-
                # broadcast on GPSIMD (idle engine), multiply on DVE. Short
                # per-hh chains so the pv pool slot frees ASAP.
                for hh in range(2):
                    oh = hh * 64
                    den = recp.tile([1, 512], F32, tag="rec", name="den")
                    nc.scalar.activation(den, pv[hh][64:65, :], AF.Ln)
                    nc.scalar.activation(den, den, AF.Exp, scale=-1.0)
                    bcs = bcsp.tile([64, 512], F32, tag="bcs", name="bcs")
                    nc.gpsimd.partition_broadcast(bcs, den, channels=64)
                    nc.vector.tensor_mul(otn[oh:oh + 64, p, qsl],
                                         pv[hh][0:64, :], bcs)
            # output projection for this q-block becomes filler work
            for t in range(4 * j, 4 * j + 4):
                for n in range(2):
                    queue.append((("o", t, n), mk(emit_outproj_tile, t, n)))
        # drain remaining fillers (late out-projs); attention is done, so
        # alternate between the ps512 and freed pvp banks to pipeline the
        # matmul/evacuate/DMA chain
        drain_i = 0
        while queue:
            key, fn = queue.popleft()
            if key[0] == "o" and drain_i % 2 == 1:
                emit_outproj_tile(key[1], key[2], pool=pvp)
            else:
                fn()
            drain_i += 1

    nc.compile()
    return nc


def _tri_np():
    # staircase causal bias: tri[kk, x] = NEG if x < 512+kk else 0, cols
    # 128:640 of the logical [128,1024] table (the only columns ever read)
    xs = np.arange(128, 640)[None, :]
    ks = np.arange(128)[:, None]
    return np.where(xs < 512 + ks, np.float32(NEG),
                    np.float32(0.0)).astype(np.float32)


def _perm2_wo(w):
    # wo rows permuted so a partition's 2 chunk-rows are DRAM-contiguous
    ch, d = w.shape
    return np.ascontiguousarray(
        w.reshape(2, ch // 2, d).transpose(1, 0, 2).reshape(ch, d))


def _perm8(w):
    # row r of the permuted layout = row (r%8)*128 + r//8 of w, so the 8
    # contraction-chunk rows a partition needs are DRAM-contiguous
    d, ch = w.shape
    return np.ascontiguousarray(
        w.reshape(8, d // 8, ch).transpose(1, 0, 2).reshape(d, ch))


def build_in_maps(x, Wq, bq, Wk, bk, Wv, bv, Wo):
    tri_np = _tri_np()
    ones_np = np.ones((1, 512), dtype=np.float32)
    xT_b = [np.ascontiguousarray(x[b].T) for b in range(B)]
    in_maps = []
    for c in range(N_CORES):
        b, tp = divmod(c, TPG)
        sl = slice(tp * CH, (tp + 1) * CH)
        wv_aug = np.zeros((D, CHA), dtype=np.float32)
        bv_aug = np.zeros((1, CHA), dtype=np.float32)
        for h in range(HPC):
            hsl = slice(tp * CH + h * DH, tp * CH + (h + 1) * DH)
            wv_aug[:, h * 65:h * 65 + DH] = Wv[:, hsl]
            bv_aug[0, h * 65:h * 65 + DH] = bv[hsl]
            bv_aug[0, h * 65 + DH] = 1.0
        in_maps.append({
            "xT": xT_b[b].astype(ml_dtypes.bfloat16),
            "wq": _perm8((Wq[:, sl].astype(np.float32)
                          * 0.125).astype(ml_dtypes.bfloat16)),
            "wk": _perm8(
                Wk[:, sl].astype(np.float32).astype(ml_dtypes.bfloat16)),
            "wv": _perm8(wv_aug.astype(ml_dtypes.bfloat16)),
            "wo": _perm2_wo(
                Wo[sl, :].astype(np.float32).astype(ml_dtypes.bfloat16)),
            "bq": (bq[sl].astype(np.float32) * 0.125).reshape(2, 128).T.copy(),
            "bk": bk[sl].astype(np.float32).reshape(2, 128).T.copy(),
            "bv": bv_aug.astype(ml_dtypes.bfloat16),
            "tri": tri_np,
            "ones": ones_np.astype(ml_dtypes.bfloat16),
            "onesf": ones_np[:, :64].copy(),
        })
    return in_maps


def _get_program():
    global _PROG
    if _PROG is None:
        _PROG = _build_program()
    return _PROG


def kernel(x, mask, Wq, bq, Wk, bk, Wv, bv, Wo, bo):
    x = np.asarray(x, dtype=np.float32)
    mask = np.asarray(mask)
    Wq, Wk, Wv, Wo = (np.asarray(w, dtype=np.float32)
                      for w in (Wq, Wk, Wv, Wo))
    bq, bk, bv, bo = (np.asarray(b, dtype=np.float32)
                      for b in (bq, bk, bv, bo))
    causal = bool(
        np.array_equal(mask != 0,
                       np.tril(np.ones((S, S), dtype=bool))))
    if not causal:
        # Fallback for non-causal masks: exact host computation.
        q = (x @ Wq + bq).reshape(B, S, H, DH).transpose(0, 2, 1, 3)
        k = (x @ Wk + bk).reshape(B, S, H, DH).transpose(0, 2, 1, 3)
        v = (x @ Wv + bv).reshape(B, S, H, DH).transpose(0, 2, 1, 3)
        attn = np.einsum("bhqd,bhkd->bhqk", q, k) / np.sqrt(np.float32(DH))
        attn = np.where(mask == 0, np.float32(-1e9), attn)
        attn = attn - attn.max(axis=-1, keepdims=True)
        e = np.exp(attn)
        p = e / e.sum(axis=-1, keepdims=True)
        o = np.einsum("bhqk,bhkd->bhqd", p, v)
        o = o.transpose(0, 2, 1, 3).reshape(B, S, D)
        return (o @ Wo + bo).astype(np.float32)

    nc = _get_program()
    in_maps = build_in_maps(x, Wq, bq, Wk, bk, Wv, bv, Wo)
    res = run_bass_kernel_spmd(nc, in_maps, core_ids=list(range(N_CORES)))
    out = np.zeros((B, S, D), dtype=np.float32)
    for c in range(N_CORES):
        out[c // TPG] += res.results[c]["out"].astype(np.float32)
    out += bo.astype(np.float32)
    return out
